# revision 1
# baseline (speedup 1.0000x reference)
"""Trainium2 Bass kernel for clamped cubic B-spline basis evaluation.

Computes, for x: [N] f32 and a clamped knot vector t (K=10, degree 3):
    z = (x - min(x)) / (max(x) - min(x) + 1e-8)
    out[n, j] = B_j^3(z[n]),  j = 0..5   -> [N, 6] f32

Strategy: trivially data-parallel over 8 NeuronCores (N/8 points each).
Per core, points stream through [128 x FD] tiles.  The Cox-de Boor
recursion is evaluated with a continuous reformulation (relu/min hats,
complementary-weight blends); all selection boundaries are continuous
crossings so the masked reference is matched to ~1e-6 without branches.

Work is balanced across three engines (PE is useless here: fp32
identity-matmul accumulation costs ~4x a DVE op on TRN2):
  - ACT (7 ops/tile): normalization (runtime scale/bias as per-partition
    APs), the two corner relu chains, and the two boundary cubes as
    exp(3*ln(.)) - all activation funcs forced into the single
    `natural_log_exp_and_others` table set so the table loads once.
  - DVE (13 ops/tile): seven fused custom DVE ops (registered at build
    time with computed uop hashes) + scalar_tensor_tensor combines.
  - GPSIMD (7 ops/tile): plain tensor_tensor products/adds (walrus
    rejects 2-stream scalar_tensor_tensor on Pool).

The tile loop is software-pipelined two-deep (stage1 = load/normalize/
blend, stage2 = output assembly/store of the previous tile) so each
engine's program order never head-of-line blocks on another engine; the
first/last tiles are split in half to shorten the fill/drain ramps.

The [N, 6] output is assembled interleaved in SBUF (stride-6 writes by
the final ops) so each DRAM store is a single contiguous DMA.

Cost-model timeline: ~143 us per core (DVE/Pool both ~88% busy; the
HBM roofline for the 28 MB/core of traffic is ~82 us).
"""

import numpy as np

N_POINTS = 8_388_608
N_CORES = 8
P = 128          # SBUF partitions
FD = 512         # free-dim elements per tile
N_SHARD = N_POINTS // N_CORES
TILE_ELEMS = P * FD
T_TILES = N_SHARD // TILE_ELEMS

_cache = {}
_ops = None


def _register_ops():
    """Register the fused custom DVE ops (idempotent)."""
    global _ops
    if _ops is not None:
        return _ops
    import concourse.dve_ops as D
    from concourse.dve_spec import Spec, Src0, Src1, C0, C1, C2, One, relu, sq, minn, lower
    from concourse.dve_uop import DveOpSpec

    def reg(name, body):
        if name in D._SUB_OPCODE_FOR_NAME:
            return next(o for o in D.OPS if o.name == name)
        spec = Spec(body=body)
        row = 1 + len(D.OPS)
        assert row < 0x20, "custom-DVE opcode rows exhausted"
        shas = {}
        for ver in ("v3", "v4"):
            tmp = DveOpSpec(
                name=name, opcode=row, uops=lower(spec, ver=ver),
                rd1_en=D.has_src1(spec),
            )
            shas[ver] = tmp.sha(ver)
        op = D.DveOp(name, spec, False, uops_sha=shas)
        D.OPS.append(op)
        D._SUB_OPCODE_FOR_NAME[name] = row
        D.CUSTOM_DVE_SPECS[name] = spec
        return op

    e = Src0 * C0 + C1
    p = Src0 * C0
    _ops = {
        # relu(min(z*c0, z*c1 + c2))                      -> B1_3
        "BSPL_HAT_A": reg("BSPL_HAT_A", relu(minn(Src0 * C0, Src0 * C1 + C2))),
        # relu(min(z*c0 + c1, (1-z)*c2))                  -> B1_4 (scaled)
        "BSPL_HAT_B": reg("BSPL_HAT_B", relu(minn(Src0 * C0 + C1, (One - Src0) * C2))),
        # relu(z*c0 + c1)^3                               -> B3_0 / B3_5
        "BSPL_CUBE": reg("BSPL_CUBE", (lambda t: sq(t) * t)(relu(e))),
        # (1-e)*relu(e)^2 = E*B2_5                        -> OUT4 partial
        "BSPL_ECORN": reg("BSPL_ECORN", (One - e) * sq(relu(e))),
        # p*relu(1-p)^2 = p*B2_1                          -> OUT1 partial
        "BSPL_PCORN": reg("BSPL_PCORN", p * sq(relu(One - p))),
        # relu(e - e^2) + (z - c2)*Src1 = E*B1_5 + h*B1_4 -> B2_4
        "BSPL_ADDRAMP": reg("BSPL_ADDRAMP", relu(e - sq(e)) + (Src0 - C2) * Src1),
        # p*relu(1-p) + (1 - z*c1)*Src1 = p*B1_2 + M*B1_3 -> B2_2
        "BSPL_BLEND2": reg("BSPL_BLEND2", p * relu(One - p) + (One - Src0 * C1) * Src1),
        # (c0 - z)*Src1                                   -> H*B1_4 etc.
        "BSPL_RAMPSUB": reg("BSPL_RAMPSUB", (C0 - Src0) * Src1),
    }
    return _ops


def _build(c1, c2, act_cube=True):
    """Build + compile the per-core Bass program. c1, c2: interior knots."""
    import concourse.bacc as bacc
    import concourse.mybir as mybir
    import concourse.tile as tile

    ops = _register_ops()
    f32 = mybir.dt.float32
    AF = mybir.ActivationFunctionType
    ALU = mybir.AluOpType

    nc = bacc.Bacc("TRN2", target_bir_lowering=False, debug=False)
    x_d = nc.dram_tensor("x", [T_TILES, P, FD], f32, kind="ExternalInput")
    st_d = nc.dram_tensor("stats", [P, 4], f32, kind="ExternalInput")
    o_d = nc.dram_tensor("out", [T_TILES, P, FD * 6], f32, kind="ExternalOutput")
    x_ap, st_ap, o_ap = x_d.ap(), st_d.ap(), o_d.ap()

    rc1 = 1.0 / c1
    rc2 = 1.0 / c2
    rdc = 1.0 / (c2 - c1)
    rg1 = 1.0 / (1.0 - c1)
    rg2 = 1.0 / (1.0 - c2)

    def cust(op, out, in0, s0=0.0, s1=0.0, imm2=0.0, in1=None):
        nc.vector._custom_dve(ops[op], out=out, in0=in0, in1=in1,
                              s0=s0, s1=s1, imm2=imm2)

    with tile.TileContext(nc) as tc:
        with (
            tc.tile_pool(name="io", bufs=3) as io,
            tc.tile_pool(name="wk", bufs=2) as wk,
            tc.tile_pool(name="wks", bufs=2) as wks,
            tc.tile_pool(name="cst", bufs=1) as cst,
        ):
            st = cst.tile([P, 4], f32, tag="st", name="st")
            nc.sync.dma_start(st[:], st_ap[:])
            s_ap = st[:, 0:1]
            b_ap = st[:, 1:2]
            b15_ap = st[:, 2:3]

            deep = {"z", "b13", "b14s", "b22", "b23", "b24",
                    "zb23", "zb24"}

            def wt(tag, w=FD):
                pool = wk if tag in deep else wks
                return pool.tile([P, FD], f32, tag=tag, name=tag)[:, :w]

            def stage1(t, lo=0, w=FD):
                """Load + normalize + hats/blends + products for tile t."""
                h = {"lo": lo, "w": w}
                xt = io.tile([P, FD], f32, tag="x", name="x")[:, :w]
                nc.sync.dma_start(xt[:], x_ap[t][:, lo:lo + w])

                z = wt("z", w)
                # z >= 0 by construction, so Relu == affine here (Copy
                # does not accept an AP bias).
                nc.scalar.activation(z[:], xt[:], AF.Relu, bias=b_ap, scale=s_ap)
                b12 = wt("b12", w)
                nc.scalar.activation(b12[:], z[:], AF.Relu, bias=1.0, scale=-rc1)
                b15 = wt("b15", w)
                nc.scalar.activation(b15[:], z[:], AF.Relu, bias=b15_ap, scale=rg2)

                b13 = wt("b13", w)
                cust("BSPL_HAT_A", b13[:], z[:], rc1, -rdc, c2 * rdc)
                b14s = wt("b14s", w)   # rg1 * B1_4
                cust("BSPL_HAT_B", b14s[:], z[:], rdc * rg1, -c1 * rdc * rg1, rg2 * rg1)
                b22 = wt("b22", w)     # B2_2 = p*B1_2 + M*B1_3
                cust("BSPL_BLEND2", b22[:], z[:], rc1, rc2, in1=b13[:])
                b24 = wt("b24", w)     # B2_4 = E*B1_5 + h*B1_4
                cust("BSPL_ADDRAMP", b24[:], z[:], rg2, -c2 * rg2, c1, in1=b14s[:])
                pc = wt("pc", w)        # p * B2_1
                cust("BSPL_PCORN", pc[:], z[:], rc1)
                ec = wt("ec", w)        # E * B2_5
                cust("BSPL_ECORN", ec[:], z[:], rg2, -c2 * rg2)

                zb13s = wt("zb13s", w)  # m * B1_3
                nc.vector.scalar_tensor_tensor(zb13s[:], z[:], rc2, b13[:], ALU.mult, ALU.mult)
                mz22n = wt("mz22n", w)  # -m * B2_2
                nc.vector.scalar_tensor_tensor(mz22n[:], z[:], -rc2, b22[:], ALU.mult, ALU.mult)

                t23 = wt("t23", w)      # H*B1_4 = (1-z) * b14s
                cust("BSPL_RAMPSUB", t23[:], z[:], 1.0, in1=b14s[:])
                b23 = wt("b23", w)      # B2_3 = m*B1_3 + H*B1_4
                nc.gpsimd.tensor_tensor(b23[:], zb13s[:], t23[:], ALU.add)

                zb23 = wt("zb23", w)    # z * B2_3
                nc.gpsimd.tensor_tensor(zb23[:], z[:], b23[:], ALU.mult)
                t2 = wt("t2", w)        # G*B2_3 = B2_3 - z*B2_3
                nc.gpsimd.tensor_tensor(t2[:], b23[:], zb23[:], ALU.subtract)
                zb24 = wt("zb24", w)    # z * B2_4
                nc.gpsimd.tensor_tensor(zb24[:], z[:], b24[:], ALU.mult)

                ot = io.tile([P, FD * 6], f32, tag="o", name="o")[:, :w * 6]
                t1 = wt("t1", w)        # M*B2_2 = B2_2 + (-m*B2_2)
                nc.gpsimd.tensor_tensor(t1[:], b22[:], mz22n[:], ALU.add)
                # OUT1 = p*B2_1 + M*B2_2
                nc.gpsimd.tensor_tensor(ot[:, 1::6], pc[:], t1[:], ALU.add)

                h.update(z=z, b12=b12, b15=b15, b22=b22, b23=b23, b24=b24, t2=t2,
                         pc=pc, ec=ec, mz22n=mz22n, zb23=zb23, zb24=zb24, ot=ot)
                return h

            def stage2(t, h):
                """Output assembly + store for tile t."""
                ot = h["ot"]
                lo, w = h["lo"], h["w"]

                ln2 = wt("ln2", w)
                nc.scalar.activation(ln2[:], h["b12"][:], AF.Ln)
                nc.scalar.activation(ot[:, 0::6], ln2[:], AF.Exp, scale=3.0)
                ln5 = wt("ln5", w)
                nc.scalar.activation(ln5[:], h["b15"][:], AF.Ln)
                nc.scalar.activation(ot[:, 5::6], ln5[:], AF.Exp, scale=3.0)

                # OUT2 = m*B2_2 + G*B2_3 = t2 - (-m*B2_2)
                nc.gpsimd.tensor_tensor(ot[:, 2::6], h["t2"][:], h["mz22n"][:], ALU.subtract)

                t3 = wt("t3", w)        # (1-z) * B2_4
                nc.vector.scalar_tensor_tensor(t3[:], h["zb24"][:], -1.0, h["b24"][:], ALU.mult, ALU.add)
                # OUT3 = z*B2_3 + H*B2_4
                nc.vector.scalar_tensor_tensor(ot[:, 3::6], t3[:], rg1, h["zb23"][:], ALU.mult, ALU.add)
                t4 = wt("t4", w)        # E*B2_5 - c1*rg1*B2_4
                nc.vector.scalar_tensor_tensor(t4[:], h["b24"][:], -c1 * rg1, h["ec"][:], ALU.mult, ALU.add)
                # OUT4 = h*B2_4 + E*B2_5
                nc.vector.scalar_tensor_tensor(ot[:, 4::6], h["zb24"][:], rg1, t4[:], ALU.mult, ALU.add)

                nc.sync.dma_start(o_ap[t][:, lo * 6:(lo + w) * 6], ot[:])

            # two-stage software pipeline over tile units; the first and
            # last tiles are split in half to shorten the fill/drain ramps.
            units = (
                [(0, 0, FD // 2), (0, FD // 2, FD // 2)]
                + [(t, 0, FD) for t in range(1, T_TILES - 1)]
                + [(T_TILES - 1, 0, FD // 2), (T_TILES - 1, FD // 2, FD // 2)]
            )
            prev = None
            for u in units:
                h = stage1(*u)
                if prev is not None:
                    stage2(prev[0][0], prev[1])
                prev = (u, h)
            stage2(prev[0][0], prev[1])

    # Force every activation onto the one table set that covers
    # relu/ln/exp/square, so the table is loaded once instead of
    # thrashing between per-function sets (~2.7us per switch).
    import concourse.hw_specs as hw_specs
    import concourse.bacc as bacc_mod
    _orig_gat = hw_specs.get_activation_tables
    _one = "natural_log_exp_and_others"

    def _gat(arch):
        t = _orig_gat(arch)
        assert _one in t
        return {k: (v if k == _one else set()) for k, v in t.items()}

    hw_specs.get_activation_tables = _gat
    bacc_patch = getattr(bacc_mod, "get_activation_tables", None)
    if bacc_patch is not None:
        bacc_mod.get_activation_tables = _gat
    try:
        nc.compile()
    finally:
        hw_specs.get_activation_tables = _orig_gat
        if bacc_patch is not None:
            bacc_mod.get_activation_tables = bacc_patch
    return nc


def _get_compiled(knots):
    key = knots.tobytes()
    if key not in _cache:
        t = knots.astype(np.float64)
        ok = (
            knots.shape == (10,)
            and np.all(t[:4] == t[0])
            and np.all(t[6:] == t[9])
            and t[0] == 0.0
            and t[9] == 1.0
            and t[0] < t[4] < t[5] < t[9]
        )
        if not ok:
            _cache[key] = None
        else:
            _cache[key] = _build(float(t[4]), float(t[5]))
    return _cache[key]


def _reference_fallback(x, knots):
    """Numpy mirror of the jax reference, used only for unexpected knots."""
    t = knots.astype(np.float32)
    K = t.shape[0]
    xmin, xmax = x.min(), x.max()
    d = np.float32(np.float32(xmax - xmin) + np.float32(1e-8))
    z = ((x - xmin) / d).astype(np.float32)[:, None]
    left, right = t[None, :-1], t[None, 1:]
    B = ((z >= left) & (z < right)).astype(np.float32)
    B = np.where((z == t[-1]) & (right == t[-1]) & (left < right), np.float32(1.0), B)
    for dgr in range(1, 4):
        tL, tLd = t[: K - dgr - 1], t[dgr : K - 1]
        tR, tRd = t[1 : K - dgr], t[dgr + 1 : K]
        den1, den2 = tLd - tL, tRd - tR
        safe1 = np.where(den1 > 0, den1, 1.0).astype(np.float32)
        safe2 = np.where(den2 > 0, den2, 1.0).astype(np.float32)
        w1 = np.where(den1[None] > 0, (z - tL[None]) / safe1[None], 0.0).astype(np.float32)
        w2 = np.where(den2[None] > 0, (tRd[None] - z) / safe2[None], 0.0).astype(np.float32)
        B = (w1 * B[:, :-1] + w2 * B[:, 1:]).astype(np.float32)
    return B


def kernel(x, knots):
    from concourse import bass_utils

    x = np.ascontiguousarray(np.asarray(x, dtype=np.float32).ravel())
    knots = np.ascontiguousarray(np.asarray(knots, dtype=np.float32).ravel())
    assert x.shape[0] == N_POINTS, x.shape

    nc = _get_compiled(knots)
    if nc is None:  # unexpected knot structure: safe host fallback
        return _reference_fallback(x, knots)

    xmin = x.min()
    xmax = x.max()
    d = np.float32(np.float32(xmax - xmin) + np.float32(1e-8))
    s = np.float32(1.0) / d
    b = np.float32(-(xmin * s))
    c2f = np.float64(knots[5])
    stats = np.empty((P, 4), np.float32)
    stats[:, 0] = s
    stats[:, 1] = b
    stats[:, 2] = np.float32(-c2f / (1.0 - c2f))
    stats[:, 3] = 0.0

    shards = x.reshape(N_CORES, T_TILES, P, FD)
    in_maps = [{"x": shards[i], "stats": stats} for i in range(N_CORES)]
    res = bass_utils.run_bass_kernel_spmd(nc, in_maps, list(range(N_CORES)))
    out = np.empty((N_CORES, N_SHARD * 6), np.float32)
    for i in range(N_CORES):
        out[i] = res.results[i]["out"].reshape(-1)
    return out.reshape(N_POINTS, 6)



# revision 2
# speedup vs baseline: 1.9472x; 1.9472x over previous
"""Trainium2 Bass kernel for clamped cubic B-spline basis evaluation.

Computes, for x: [N] f32 and a clamped knot vector t (K=10, degree 3):
    z = (x - min(x)) / (max(x) - min(x) + 1e-8)
    out[n, j] = B_j^3(z[n]),  j = 0..5   -> [N, 6] f32

Math: on [0, 1] every B_j is an exact linear combination of the
truncated-power basis {1, z, z^2, z^3, relu(z-c1)^3, relu(z-c2)^3}
(c1, c2 = interior knots).  The clamped structure makes the combos tiny:
    B5 = relu((z-c2)/(1-c2))^3             B0 = relu((c1-z)/c1)^3
    B4 = e4*E1 + f4*E2                     B1 = mirrored
    B3 = d3*z^3 + e3*E1 + f3*E2            B2 = mirrored
with E1 = relu(z-c1)^3, E2 = relu(z-c2)^3.  B3 reuses the B4 plane:
    B3 = cube(d3^{1/3} z) + m*(B4 + (n/m)*B5),  m = e3/e4.

Engine mapping per [128 x FD] tile (costs per the TimelineSim model):
  ACT   (5 ops): z = Relu(s*x+b) [f32] and the two corner relu/square
        pairs (fp16) - all affine-from-x with per-partition scale/bias.
  Pool  (2 ops): corner cube mults  B5 = r5q*r5, B0 = r0q*r0 (fp16).
  DVE   (8 ops): two 7-stage fused customs (B4, B1: relu-cube + stream
        term), two cube+stream customs (B3, B2), plus two fp16
        tensor_scalar (4x mode) and two fp16 tensor_tensor (2x mode)
        for the plane combines.
All six outputs are written as separate fp16 planes (contiguous DMA,
half the store traffic of f32); the host interleaves/upcasts.  Inputs
are loaded as fp16 (half the load traffic); worst-case added error
~1e-3, well inside the 2e-2 gate.
"""

import numpy as np

N_POINTS = 8_388_608
N_CORES = 8
P = 128          # SBUF partitions
FD = 2048        # free-dim elements per tile
N_SHARD = N_POINTS // N_CORES
TILE_ELEMS = P * FD
T_TILES = N_SHARD // TILE_ELEMS

_cache = {}
_ops = None


def _register_ops():
    """Register the fused custom DVE ops (idempotent)."""
    global _ops
    if _ops is not None:
        return _ops
    import concourse.dve_ops as D
    from concourse.dve_spec import Spec, Src0, Src1, C0, C1, C2, relu, sq, lower
    from concourse.dve_uop import DveOpSpec

    def reg(name, body):
        if name in D._SUB_OPCODE_FOR_NAME:
            return next(o for o in D.OPS if o.name == name)
        spec = Spec(body=body)
        row = 1 + len(D.OPS)
        assert row < 0x20, "custom-DVE opcode rows exhausted"
        shas = {}
        for ver in ("v3", "v4"):
            tmp = DveOpSpec(
                name=name, opcode=row, uops=lower(spec, ver=ver),
                rd1_en=D.has_src1(spec),
            )
            shas[ver] = tmp.sha(ver)
        op = D.DveOp(name, spec, False, uops_sha=shas)
        D.OPS.append(op)
        D._SUB_OPCODE_FOR_NAME[name] = row
        D.CUSTOM_DVE_SPECS[name] = spec
        return op

    def cube(t):
        return sq(t) * t

    def rcube(t):
        r = relu(t)
        return sq(r) * r

    _ops = {
        # relu(C0*z + C2)^3 - C1*in1        -> B4 / B1
        "BSPL_RCS1": reg("BSPL_RCS1", rcube(C0 * Src0 + C2) - C1 * Src1),
        # (C1 - C0*z)^3 - C2*in1            -> B3 / B2
        "BSPL_CBS1": reg("BSPL_CBS1", cube(C1 - C0 * Src0) - C2 * Src1),
    }
    return _ops


def _tp_coeffs(c1, c2):
    """Truncated-power coefficients of the 6 basis cubics for knots
    [0,0,0,0,c1,c2,1,1,1,1], via a float64 lstsq fit on
    {1, z, z^2, z^3, relu(z-c1)^3, relu(z-c2)^3}.  Returns the [6, 6]
    matrix (rows = features, cols = B0..B5) or None if the fit is bad."""
    t = np.array([0, 0, 0, 0, c1, c2, 1, 1, 1, 1], np.float64)
    K = 10
    zs = np.linspace(1e-4, 1 - 1e-4, 4001)[:, None]
    left, right = t[None, :-1], t[None, 1:]
    B = ((zs >= left) & (zs < right)).astype(np.float64)
    for d in range(1, 4):
        tL, tLd = t[: K - d - 1], t[d : K - 1]
        tR, tRd = t[1 : K - d], t[d + 1 : K]
        den1, den2 = tLd - tL, tRd - tR
        s1 = np.where(den1 > 0, den1, 1.0)
        s2 = np.where(den2 > 0, den2, 1.0)
        w1 = np.where(den1[None] > 0, (zs - tL[None]) / s1[None], 0.0)
        w2 = np.where(den2[None] > 0, (tRd[None] - zs) / s2[None], 0.0)
        B = w1 * B[:, :-1] + w2 * B[:, 1:]
    z = zs[:, 0]
    Phi = np.stack([np.ones_like(z), z, z * z, z**3,
                    np.maximum(z - c1, 0.0) ** 3,
                    np.maximum(z - c2, 0.0) ** 3], 1)
    M, *_ = np.linalg.lstsq(Phi, B, rcond=None)
    if not np.isfinite(M).all() or np.abs(Phi @ M - B).max() > 1e-9:
        return None
    return M


def _plan(c1, c2):
    """Solve for all compile-time constants.  Returns dict or None."""
    M = _tp_coeffs(c1, c2)
    Mm = _tp_coeffs(1.0 - c2, 1.0 - c1)   # reflected knots, for B1/B2
    if M is None or Mm is None:
        return None
    # sparsity asserts: B4 = e4*E1 + f4*E2, B3 = d3*z^3 + e3*E1 + f3*E2
    if np.abs(M[:4, 4]).max() > 1e-7 or np.abs(M[:3, 3]).max() > 1e-7:
        return None
    if np.abs(Mm[:4, 4]).max() > 1e-7 or np.abs(Mm[:3, 3]).max() > 1e-7:
        return None
    e4, f4 = M[4, 4], M[5, 4]
    d3, e3, f3 = M[3, 3], M[4, 3], M[5, 3]
    e4m, f4m = Mm[4, 4], Mm[5, 4]
    d3m, e3m, f3m = Mm[3, 3], Mm[4, 3], Mm[5, 3]
    if min(e4, d3, e4m, d3m) <= 0 or abs(e3) < 1e-12 or abs(e3m) < 1e-12:
        return None
    m = e3 / e4
    n = (f3 - m * f4) * (1.0 - c2) ** 3
    mm = e3m / e4m
    nm = (f3m - mm * f4m) * c1**3
    return {
        # B4 custom: rcube(C0*z + C2) - C1*p5
        "b4": (e4 ** (1 / 3), -f4 * (1 - c2) ** 3, -(e4 ** (1 / 3)) * c1),
        # B1 custom: rcube(C0*z + C2) - C1*p0   (C0 negative: relu(c2-z))
        "b1": (-(e4m ** (1 / 3)), -f4m * c1**3, (e4m ** (1 / 3)) * c2),
        # B3 custom: (C1 - C0*z)^3 - C2*w5 ; w5 = p4 + ts5*p5
        "b3": (-(d3 ** (1 / 3)), 0.0, -m),
        "ts5": n / m,
        # B2 custom: (C1 - C0*z)^3 - C2*w0 ; w0 = p1 + ts0*p0
        "b2": (d3m ** (1 / 3), d3m ** (1 / 3), -mm),
        "ts0": nm / mm,
    }


def _build(c1, c2):
    """Build + compile the per-core Bass program for interior knots c1<c2."""
    import concourse.bacc as bacc
    import concourse.mybir as mybir
    import concourse.tile as tile

    plan = _plan(c1, c2)
    if plan is None:
        return None
    ops = _register_ops()
    f32 = mybir.dt.float32
    f16 = mybir.dt.float16
    AF = mybir.ActivationFunctionType
    ALU = mybir.AluOpType

    nc = bacc.Bacc("TRN2", target_bir_lowering=False, debug=False)
    x_d = nc.dram_tensor("x", [T_TILES, P, FD], f16, kind="ExternalInput")
    st_d = nc.dram_tensor("stats", [P, 8], f32, kind="ExternalInput")
    o_d = nc.dram_tensor("out", [T_TILES, 6, P, FD], f16, kind="ExternalOutput")
    x_ap, st_ap, o_ap = x_d.ap(), st_d.ap(), o_d.ap()

    c04, c14, c24 = plan["b4"]
    c01, c11, c21 = plan["b1"]
    c03, c13, c23 = plan["b3"]
    c02, c12, c22 = plan["b2"]
    ts5, ts0 = plan["ts5"], plan["ts0"]

    def cust(op, out, in0, in1, s0, s1, imm2):
        nc.vector._custom_dve(ops[op], out=out, in0=in0, in1=in1,
                              s0=s0, s1=s1, imm2=imm2)

    with tile.TileContext(nc) as tc:
        with (
            tc.tile_pool(name="io", bufs=3) as io,
            tc.tile_pool(name="wk", bufs=2) as wk,
            tc.tile_pool(name="cst", bufs=1) as cst,
        ):
            st = cst.tile([P, 8], f32, tag="st", name="st")
            nc.sync.dma_start(st[:], st_ap[:])
            sz_ap = st[:, 0:1]    # z scale
            bz_ap = st[:, 1:2]    # z bias
            a5_ap = st[:, 2:3]    # r5 scale
            b5_ap = st[:, 3:4]    # r5 bias
            a0_ap = st[:, 4:5]    # r0 scale
            b0_ap = st[:, 5:6]    # r0 bias

            for t in range(T_TILES):
                xt = io.tile([P, FD], f16, tag="x", name="x")
                nc.sync.dma_start(xt[:], x_ap[t])

                z = wk.tile([P, FD], f32, tag="z", name="z")
                nc.scalar.activation(z[:], xt[:], AF.Relu, bias=bz_ap, scale=sz_ap)
                r5 = wk.tile([P, FD], f16, tag="r5", name="r5")
                nc.scalar.activation(r5[:], xt[:], AF.Relu, bias=b5_ap, scale=a5_ap)
                r5q = wk.tile([P, FD], f16, tag="r5q", name="r5q")
                nc.scalar.activation(r5q[:], r5[:], AF.Square)
                r0 = wk.tile([P, FD], f16, tag="r0", name="r0")
                nc.scalar.activation(r0[:], xt[:], AF.Relu, bias=b0_ap, scale=a0_ap)
                r0q = wk.tile([P, FD], f16, tag="r0q", name="r0q")
                nc.scalar.activation(r0q[:], r0[:], AF.Square)

                p5 = io.tile([P, FD], f16, tag="p5", name="p5")
                nc.gpsimd.tensor_tensor(p5[:], r5q[:], r5[:], ALU.mult)
                p0 = io.tile([P, FD], f16, tag="p0", name="p0")
                nc.gpsimd.tensor_tensor(p0[:], r0q[:], r0[:], ALU.mult)

                p4 = io.tile([P, FD], f16, tag="p4", name="p4")
                cust("BSPL_RCS1", p4[:], z[:], p5[:], c04, c14, c24)
                p1 = io.tile([P, FD], f16, tag="p1", name="p1")
                cust("BSPL_RCS1", p1[:], z[:], p0[:], c01, c11, c21)

                v5 = wk.tile([P, FD], f16, tag="v5", name="v5")
                nc.vector.tensor_scalar(v5[:], p5[:], float(ts5), None, op0=ALU.mult)
                w5 = wk.tile([P, FD], f16, tag="w5", name="w5")
                nc.vector.tensor_tensor(w5[:], p4[:], v5[:], ALU.add)
                p3 = io.tile([P, FD], f16, tag="p3", name="p3")
                cust("BSPL_CBS1", p3[:], z[:], w5[:], c03, c13, c23)

                v0 = wk.tile([P, FD], f16, tag="v0", name="v0")
                nc.vector.tensor_scalar(v0[:], p0[:], float(ts0), None, op0=ALU.mult)
                w0 = wk.tile([P, FD], f16, tag="w0", name="w0")
                nc.vector.tensor_tensor(w0[:], p1[:], v0[:], ALU.add)
                p2 = io.tile([P, FD], f16, tag="p2", name="p2")
                cust("BSPL_CBS1", p2[:], z[:], w0[:], c02, c12, c22)

                for j, pl in enumerate((p0, p1, p2, p3, p4, p5)):
                    nc.sync.dma_start(o_ap[t][j], pl[:])

    nc.compile()
    return nc


def _get_compiled(knots):
    key = knots.tobytes()
    if key not in _cache:
        t = knots.astype(np.float64)
        ok = (
            knots.shape == (10,)
            and np.all(t[:4] == t[0])
            and np.all(t[6:] == t[9])
            and t[0] == 0.0
            and t[9] == 1.0
            and t[0] < t[4] < t[5] < t[9]
        )
        if not ok:
            _cache[key] = None
        else:
            _cache[key] = _build(float(t[4]), float(t[5]))
    return _cache[key]


def _reference_fallback(x, knots):
    """Numpy mirror of the jax reference, used only for unexpected knots."""
    t = knots.astype(np.float32)
    K = t.shape[0]
    xmin, xmax = x.min(), x.max()
    d = np.float32(np.float32(xmax - xmin) + np.float32(1e-8))
    z = ((x - xmin) / d).astype(np.float32)[:, None]
    left, right = t[None, :-1], t[None, 1:]
    B = ((z >= left) & (z < right)).astype(np.float32)
    B = np.where((z == t[-1]) & (right == t[-1]) & (left < right), np.float32(1.0), B)
    for dgr in range(1, 4):
        tL, tLd = t[: K - dgr - 1], t[dgr : K - 1]
        tR, tRd = t[1 : K - dgr], t[dgr + 1 : K]
        den1, den2 = tLd - tL, tRd - tR
        safe1 = np.where(den1 > 0, den1, 1.0).astype(np.float32)
        safe2 = np.where(den2 > 0, den2, 1.0).astype(np.float32)
        w1 = np.where(den1[None] > 0, (z - tL[None]) / safe1[None], 0.0).astype(np.float32)
        w2 = np.where(den2[None] > 0, (tRd[None] - z) / safe2[None], 0.0).astype(np.float32)
        B = (w1 * B[:, :-1] + w2 * B[:, 1:]).astype(np.float32)
    return B


def kernel(x, knots):
    from concourse import bass_utils

    x = np.ascontiguousarray(np.asarray(x, dtype=np.float32).ravel())
    knots = np.ascontiguousarray(np.asarray(knots, dtype=np.float32).ravel())
    assert x.shape[0] == N_POINTS, x.shape

    nc = _get_compiled(knots)
    if nc is None:  # unexpected knot structure: safe host fallback
        return _reference_fallback(x, knots)

    kd = knots.astype(np.float64)
    c1, c2 = float(kd[4]), float(kd[5])
    xmin = x.min()
    xmax = x.max()
    d = np.float32(np.float32(xmax - xmin) + np.float32(1e-8))
    s = float(np.float32(1.0) / d)
    b = float(np.float32(-(xmin * s)))
    stats = np.zeros((P, 8), np.float32)
    stats[:, 0] = s
    stats[:, 1] = b
    stats[:, 2] = s / (1.0 - c2)          # r5 = relu((z-c2)/(1-c2))
    stats[:, 3] = (b - c2) / (1.0 - c2)
    stats[:, 4] = -s / c1                 # r0 = relu((c1-z)/c1)
    stats[:, 5] = (c1 - b) / c1

    xh = x.astype(np.float16).reshape(N_CORES, T_TILES, P, FD)
    in_maps = [{"x": xh[i], "stats": stats} for i in range(N_CORES)]
    res = bass_utils.run_bass_kernel_spmd(nc, in_maps, list(range(N_CORES)))
    out = np.empty((N_CORES, T_TILES, P, FD, 6), np.float32)
    for i in range(N_CORES):
        o = res.results[i]["out"]          # [T, 6, P, FD] fp16
        out[i] = np.asarray(o, np.float16).transpose(0, 2, 3, 1).astype(np.float32)
    return out.reshape(N_POINTS, 6)


# revision 4
# speedup vs baseline: 2.1272x; 1.0924x over previous
"""Trainium2 Bass kernel for clamped cubic B-spline basis evaluation.

Computes, for x: [N] f32 and a clamped knot vector t (K=10, degree 3):
    z = (x - min(x)) / (max(x) - min(x) + 1e-8)
    out[n, j] = B_j^3(z[n]),  j = 0..5   -> [N, 6] f32

Math: on [0, 1] every B_j is an exact linear combination of the
truncated-power basis {1, z, z^2, z^3, relu(z-c1)^3, relu(z-c2)^3}
(c1, c2 = interior knots).  The clamped structure makes the combos tiny:
    B5 = relu((z-c2)/(1-c2))^3             B0 = relu((c1-z)/c1)^3
    B4 = e4*E1 + f4*E2                     B1 = mirrored
    B3 = d3*z^3 + e3*E1 + f3*E2            B2 = mirrored
with E1 = relu(z-c1)^3, E2 = relu(z-c2)^3.  B3 reuses the B4 plane:
    B3 = cube(d3^{1/3} z) + m*(B4 + (n/m)*B5),  m = e3/e4.

Engine mapping per [128 x FD] tile (costs per the TimelineSim model):
  ACT   (5 ops): z = Relu(s*x+b) [f32] and the two corner relu/square
        pairs (fp16) - all affine-from-x with per-partition scale/bias.
  Pool  (2 ops): corner cube mults  B5 = r5q*r5, B0 = r0q*r0 (fp16).
  DVE   (8 ops): two 7-stage fused customs (B4, B1: relu-cube + stream
        term), two cube+stream customs (B3, B2), plus two fp16
        tensor_scalar (4x mode) and two fp16 tensor_tensor (2x mode)
        for the plane combines.
All six outputs are written as separate fp16 planes (contiguous DMA,
half the store traffic of f32); the host interleaves/upcasts.  Inputs
are loaded as fp16 (half the load traffic); worst-case added error
~1e-3, well inside the 2e-2 gate.
"""

import numpy as np

N_POINTS = 8_388_608
N_CORES = 8
P = 128          # SBUF partitions
FD = 1024        # free-dim elements per tile
N_SHARD = N_POINTS // N_CORES
TILE_ELEMS = P * FD
T_TILES = N_SHARD // TILE_ELEMS

_cache = {}
_ops = None


def _register_ops():
    """Register the fused custom DVE ops (idempotent)."""
    global _ops
    if _ops is not None:
        return _ops
    import concourse.dve_ops as D
    from concourse.dve_spec import Spec, Src0, Src1, C0, C1, C2, relu, sq, lower
    from concourse.dve_uop import DveOpSpec

    def reg(name, body):
        if name in D._SUB_OPCODE_FOR_NAME:
            return next(o for o in D.OPS if o.name == name)
        spec = Spec(body=body)
        row = 1 + len(D.OPS)
        assert row < 0x20, "custom-DVE opcode rows exhausted"
        shas = {}
        for ver in ("v3", "v4"):
            tmp = DveOpSpec(
                name=name, opcode=row, uops=lower(spec, ver=ver),
                rd1_en=D.has_src1(spec),
            )
            shas[ver] = tmp.sha(ver)
        op = D.DveOp(name, spec, False, uops_sha=shas)
        D.OPS.append(op)
        D._SUB_OPCODE_FOR_NAME[name] = row
        D.CUSTOM_DVE_SPECS[name] = spec
        return op

    def cube(t):
        return sq(t) * t

    def rcube(t):
        r = relu(t)
        return sq(r) * r

    _ops = {
        # relu(C0*z + C2)^3 - C1*in1        -> B4 / B1
        "BSPL_RCS1": reg("BSPL_RCS1", rcube(C0 * Src0 + C2) - C1 * Src1),
        # (C1 - C0*z)^3 - C2*in1            -> B3 / B2
        "BSPL_CBS1": reg("BSPL_CBS1", cube(C1 - C0 * Src0) - C2 * Src1),
    }
    return _ops


def _tp_coeffs(c1, c2):
    """Truncated-power coefficients of the 6 basis cubics for knots
    [0,0,0,0,c1,c2,1,1,1,1], via a float64 lstsq fit on
    {1, z, z^2, z^3, relu(z-c1)^3, relu(z-c2)^3}.  Returns the [6, 6]
    matrix (rows = features, cols = B0..B5) or None if the fit is bad."""
    t = np.array([0, 0, 0, 0, c1, c2, 1, 1, 1, 1], np.float64)
    K = 10
    zs = np.linspace(1e-4, 1 - 1e-4, 4001)[:, None]
    left, right = t[None, :-1], t[None, 1:]
    B = ((zs >= left) & (zs < right)).astype(np.float64)
    for d in range(1, 4):
        tL, tLd = t[: K - d - 1], t[d : K - 1]
        tR, tRd = t[1 : K - d], t[d + 1 : K]
        den1, den2 = tLd - tL, tRd - tR
        s1 = np.where(den1 > 0, den1, 1.0)
        s2 = np.where(den2 > 0, den2, 1.0)
        w1 = np.where(den1[None] > 0, (zs - tL[None]) / s1[None], 0.0)
        w2 = np.where(den2[None] > 0, (tRd[None] - zs) / s2[None], 0.0)
        B = w1 * B[:, :-1] + w2 * B[:, 1:]
    z = zs[:, 0]
    Phi = np.stack([np.ones_like(z), z, z * z, z**3,
                    np.maximum(z - c1, 0.0) ** 3,
                    np.maximum(z - c2, 0.0) ** 3], 1)
    M, *_ = np.linalg.lstsq(Phi, B, rcond=None)
    if not np.isfinite(M).all() or np.abs(Phi @ M - B).max() > 1e-9:
        return None
    return M


def _plan(c1, c2):
    """Solve for all compile-time constants.  Returns dict or None."""
    M = _tp_coeffs(c1, c2)
    Mm = _tp_coeffs(1.0 - c2, 1.0 - c1)   # reflected knots, for B1/B2
    if M is None or Mm is None:
        return None
    # sparsity asserts: B4 = e4*E1 + f4*E2, B3 = d3*z^3 + e3*E1 + f3*E2
    if np.abs(M[:4, 4]).max() > 1e-7 or np.abs(M[:3, 3]).max() > 1e-7:
        return None
    if np.abs(Mm[:4, 4]).max() > 1e-7 or np.abs(Mm[:3, 3]).max() > 1e-7:
        return None
    e4, f4 = M[4, 4], M[5, 4]
    d3, e3, f3 = M[3, 3], M[4, 3], M[5, 3]
    e4m, f4m = Mm[4, 4], Mm[5, 4]
    d3m, e3m, f3m = Mm[3, 3], Mm[4, 3], Mm[5, 3]
    if min(e4, d3, e4m, d3m) <= 0 or abs(e3) < 1e-12 or abs(e3m) < 1e-12:
        return None
    m = e3 / e4
    n = (f3 - m * f4) * (1.0 - c2) ** 3
    mm = e3m / e4m
    nm = (f3m - mm * f4m) * c1**3
    return {
        # B4 custom: rcube(C0*z + C2) - C1*p5
        "b4": (e4 ** (1 / 3), -f4 * (1 - c2) ** 3, -(e4 ** (1 / 3)) * c1),
        # B1 custom: rcube(C0*z + C2) - C1*p0   (C0 negative: relu(c2-z))
        "b1": (-(e4m ** (1 / 3)), -f4m * c1**3, (e4m ** (1 / 3)) * c2),
        # B3 custom: (C1 - C0*z)^3 - C2*w5 ; w5 = p4 + ts5*p5
        "b3": (-(d3 ** (1 / 3)), 0.0, -m),
        "ts5": n / m,
        # B2 custom: (C1 - C0*z)^3 - C2*w0 ; w0 = p1 + ts0*p0
        "b2": (d3m ** (1 / 3), d3m ** (1 / 3), -mm),
        "ts0": nm / mm,
    }


def _build(c1, c2):
    """Build + compile the per-core Bass program for interior knots c1<c2."""
    import concourse.bacc as bacc
    import concourse.mybir as mybir
    import concourse.tile as tile

    plan = _plan(c1, c2)
    if plan is None:
        return None
    ops = _register_ops()
    f32 = mybir.dt.float32
    f16 = mybir.dt.float16
    AF = mybir.ActivationFunctionType
    ALU = mybir.AluOpType

    nc = bacc.Bacc("TRN2", target_bir_lowering=False, debug=False)
    x_d = nc.dram_tensor("x", [T_TILES, P, FD], f16, kind="ExternalInput")
    st_d = nc.dram_tensor("stats", [P, 8], f32, kind="ExternalInput")
    o_d = nc.dram_tensor("out", [T_TILES, 6, P, FD], f16, kind="ExternalOutput")
    x_ap, st_ap, o_ap = x_d.ap(), st_d.ap(), o_d.ap()

    c04, c14, c24 = plan["b4"]
    c01, c11, c21 = plan["b1"]
    c03, c13, c23 = plan["b3"]
    c02, c12, c22 = plan["b2"]
    ts5, ts0 = plan["ts5"], plan["ts0"]

    def cust(op, out, in0, in1, s0, s1, imm2):
        nc.vector._custom_dve(ops[op], out=out, in0=in0, in1=in1,
                              s0=s0, s1=s1, imm2=imm2)

    with tile.TileContext(nc) as tc:
        with (
            tc.tile_pool(name="io", bufs=3) as io,
            tc.tile_pool(name="wk", bufs=2) as wk,
            tc.tile_pool(name="cst", bufs=1) as cst,
        ):
            st = cst.tile([P, 8], f32, tag="st", name="st")
            nc.sync.dma_start(st[:], st_ap[:])
            sz_ap = st[:, 0:1]    # z scale
            bz_ap = st[:, 1:2]    # z bias
            a5_ap = st[:, 2:3]    # r5 scale
            b5_ap = st[:, 3:4]    # r5 bias
            a0_ap = st[:, 4:5]    # r0 scale
            b0_ap = st[:, 5:6]    # r0 bias

            for t in range(T_TILES):
                xt = io.tile([P, FD], f16, tag="x", name="x")
                nc.sync.dma_start(xt[:], x_ap[t])

                # corner relu/square pairs first so Pool starts early
                r5 = wk.tile([P, FD], f16, tag="r5", name="r5")
                nc.scalar.activation(r5[:], xt[:], AF.Relu, bias=b5_ap, scale=a5_ap)
                r5q = wk.tile([P, FD], f16, tag="r5q", name="r5q")
                nc.scalar.activation(r5q[:], r5[:], AF.Square)
                r0 = wk.tile([P, FD], f16, tag="r0", name="r0")
                nc.scalar.activation(r0[:], xt[:], AF.Relu, bias=b0_ap, scale=a0_ap)
                r0q = wk.tile([P, FD], f16, tag="r0q", name="r0q")
                nc.scalar.activation(r0q[:], r0[:], AF.Square)
                z = wk.tile([P, FD], f32, tag="z", name="z")
                nc.scalar.activation(z[:], xt[:], AF.Relu, bias=bz_ap, scale=sz_ap)

                p5 = io.tile([P, FD], f16, tag="p5", name="p5")
                nc.gpsimd.tensor_tensor(p5[:], r5q[:], r5[:], ALU.mult)
                p0 = io.tile([P, FD], f16, tag="p0", name="p0")
                nc.gpsimd.tensor_tensor(p0[:], r0q[:], r0[:], ALU.mult)

                # two independent DVE chains, interleaved
                p4 = io.tile([P, FD], f16, tag="p4", name="p4")
                cust("BSPL_RCS1", p4[:], z[:], p5[:], c04, c14, c24)
                p1 = io.tile([P, FD], f16, tag="p1", name="p1")
                cust("BSPL_RCS1", p1[:], z[:], p0[:], c01, c11, c21)
                v5 = wk.tile([P, FD], f16, tag="v5", name="v5")
                nc.vector.tensor_scalar(v5[:], p5[:], float(ts5), None, op0=ALU.mult)
                v0 = wk.tile([P, FD], f16, tag="v0", name="v0")
                nc.vector.tensor_scalar(v0[:], p0[:], float(ts0), None, op0=ALU.mult)
                w5 = wk.tile([P, FD], f16, tag="w5", name="w5")
                nc.vector.tensor_tensor(w5[:], p4[:], v5[:], ALU.add)
                w0 = wk.tile([P, FD], f16, tag="w0", name="w0")
                nc.vector.tensor_tensor(w0[:], p1[:], v0[:], ALU.add)
                p3 = io.tile([P, FD], f16, tag="p3", name="p3")
                cust("BSPL_CBS1", p3[:], z[:], w5[:], c03, c13, c23)
                p2 = io.tile([P, FD], f16, tag="p2", name="p2")
                cust("BSPL_CBS1", p2[:], z[:], w0[:], c02, c12, c22)

                for j, pl in enumerate((p0, p1, p2, p3, p4, p5)):
                    nc.sync.dma_start(o_ap[t][j], pl[:])

    nc.compile()
    return nc


def _get_compiled(knots):
    key = knots.tobytes()
    if key not in _cache:
        t = knots.astype(np.float64)
        ok = (
            knots.shape == (10,)
            and np.all(t[:4] == t[0])
            and np.all(t[6:] == t[9])
            and t[0] == 0.0
            and t[9] == 1.0
            and t[0] < t[4] < t[5] < t[9]
        )
        if not ok:
            _cache[key] = None
        else:
            _cache[key] = _build(float(t[4]), float(t[5]))
    return _cache[key]


def _reference_fallback(x, knots):
    """Numpy mirror of the jax reference, used only for unexpected knots."""
    t = knots.astype(np.float32)
    K = t.shape[0]
    xmin, xmax = x.min(), x.max()
    d = np.float32(np.float32(xmax - xmin) + np.float32(1e-8))
    z = ((x - xmin) / d).astype(np.float32)[:, None]
    left, right = t[None, :-1], t[None, 1:]
    B = ((z >= left) & (z < right)).astype(np.float32)
    B = np.where((z == t[-1]) & (right == t[-1]) & (left < right), np.float32(1.0), B)
    for dgr in range(1, 4):
        tL, tLd = t[: K - dgr - 1], t[dgr : K - 1]
        tR, tRd = t[1 : K - dgr], t[dgr + 1 : K]
        den1, den2 = tLd - tL, tRd - tR
        safe1 = np.where(den1 > 0, den1, 1.0).astype(np.float32)
        safe2 = np.where(den2 > 0, den2, 1.0).astype(np.float32)
        w1 = np.where(den1[None] > 0, (z - tL[None]) / safe1[None], 0.0).astype(np.float32)
        w2 = np.where(den2[None] > 0, (tRd[None] - z) / safe2[None], 0.0).astype(np.float32)
        B = (w1 * B[:, :-1] + w2 * B[:, 1:]).astype(np.float32)
    return B


def kernel(x, knots):
    from concourse import bass_utils

    x = np.ascontiguousarray(np.asarray(x, dtype=np.float32).ravel())
    knots = np.ascontiguousarray(np.asarray(knots, dtype=np.float32).ravel())
    assert x.shape[0] == N_POINTS, x.shape

    nc = _get_compiled(knots)
    if nc is None:  # unexpected knot structure: safe host fallback
        return _reference_fallback(x, knots)

    kd = knots.astype(np.float64)
    c1, c2 = float(kd[4]), float(kd[5])
    xmin = x.min()
    xmax = x.max()
    d = np.float32(np.float32(xmax - xmin) + np.float32(1e-8))
    s = float(np.float32(1.0) / d)
    b = float(np.float32(-(xmin * s)))
    stats = np.zeros((P, 8), np.float32)
    stats[:, 0] = s
    stats[:, 1] = b
    stats[:, 2] = s / (1.0 - c2)          # r5 = relu((z-c2)/(1-c2))
    stats[:, 3] = (b - c2) / (1.0 - c2)
    stats[:, 4] = -s / c1                 # r0 = relu((c1-z)/c1)
    stats[:, 5] = (c1 - b) / c1

    xh = x.astype(np.float16).reshape(N_CORES, T_TILES, P, FD)
    in_maps = [{"x": xh[i], "stats": stats} for i in range(N_CORES)]
    res = bass_utils.run_bass_kernel_spmd(nc, in_maps, list(range(N_CORES)))
    out = np.empty((N_CORES, T_TILES, P, FD, 6), np.float32)
    for i in range(N_CORES):
        o = res.results[i]["out"]          # [T, 6, P, FD] fp16
        out[i] = np.asarray(o, np.float16).transpose(0, 2, 3, 1).astype(np.float32)
    return out.reshape(N_POINTS, 6)
